# revision 30
# baseline (speedup 1.0000x reference)
"""Trainium2 Bass kernel for causal self-attention (B=4, T=2048, C=2048, H=16).

Sharding: 16 heads across 8 cores (2 heads/core), tensor-parallel column split
of Wqkv and row split of Wout; host sums the 8 row-parallel partial outputs.

Device-side layout strategy (everything "transposed", token index in the free
dimension), which makes every matmul land in its natural layout with zero
on-chip transposes:
  qT/kT   [D=128 part, T free]   = Wq_h^T @ x  (m-tile of the qkv projection)
  V       [T part, D free]       computed with x-blocks as the stationary operand
  S^T     [k part, q free]       = kT-block^T-contraction (lhsT=kT blk, rhs=qT)
  U = exp(S^T), causal blocks skipped entirely, diagonal blocks masked via a
      triangular 0/1 multiply after exp
  y^T     [D part, q free]       = sum_kb V_blk.T @ U_blk  (PSUM accumulate)
  U_acc   [128, q]               = sum_kb U_blk            (DVE adds, bf16)
  denomR  [128 part, q free]     = ones128.T @ U_acc  (one matmul per (j,h);
            the all-ones lhsT replicates the denominator to every partition,
            so no gpsimd broadcast is needed)
  y_norm  = y^T * reciprocal_approx_fast(denomR)
  partial [tokens part, Cout]    lhsT = y^T block, rhs = Wout rows, emitted
            with a one-q-chunk lag so PE never waits on the softmax
            normalization chain; junk warm-up matmuls cover the initial DMA
            wave so HAM runs the PE at 2.4 GHz from the first real matmul

RoPE is applied in [D, T] layout: the q/k columns of Wqkv are permuted on the
host so rotation pairs land at partitions (i, i+64) ("deinterleaved"); the
half-swap is two 64-partition DVE copies, then 3 elementwise ops against
host-precomputed [128, T] cos/sin tables (the 1/sqrt(D) scale is folded into
the q tables). Scores are invariant to any fixed permutation of head dims
applied to both q and k, so the reference is reproduced exactly.
"""

import math

import numpy as np
import ml_dtypes

# Module-level knobs (test harness may set TRACE=True to capture a profile).
TRACE = False
LAST_RESULT = None  # BassKernelResults of the last run (for profiling)

_B, _T, _C, _H = 4, 2048, 2048, 16
_D = 128
_CH = 512  # free-dim chunk (one PSUM bank of fp32)


def build_program(B, T, C, COUT, HPC, n_cores=8):
    """Build the per-core Bass/Tile program (identical on all cores)."""
    import concourse.bass as bass  # noqa: F401
    import concourse.mybir as mybir
    import concourse.tile as tile
    from concourse import bacc
    from contextlib import ExitStack

    dt = mybir.dt
    f32, bf16 = dt.float32, dt.bfloat16
    D, CH = _D, _CH
    KB = C // 128          # contraction blocks for the projections
    NCH = T // CH          # 512-token chunks per batch (also q-chunks)
    NTT = T // 128         # 128-token tiles per batch (also k-tiles)
    BT = B * T
    F = HPC * D            # per-core head feature width
    AF = mybir.ActivationFunctionType

    nc = bacc.Bacc(
        "TRN2", target_bir_lowering=False, debug=False, num_devices=n_cores
    )

    # all bulk inputs are pre-blocked on the host into partition-major layout
    # so every DMA moves long contiguous runs (8-16KB descriptors instead of
    # 0.5-1KB ones; the DMA engines are descriptor-rate-limited)
    xt = nc.dram_tensor("xt", [128, B * NCH * KB * CH], bf16, kind="ExternalInput").ap()
    wq = nc.dram_tensor("wq", [128, KB * F], bf16, kind="ExternalInput").ap()
    wk = nc.dram_tensor("wk", [128, KB * F], bf16, kind="ExternalInput").ap()
    wv = nc.dram_tensor("wv", [128, KB * F], bf16, kind="ExternalInput").ap()
    wo = nc.dram_tensor("wo", [128, HPC * COUT], bf16, kind="ExternalInput").ap()
    cos64 = nc.dram_tensor("cos64", [64, T], bf16, kind="ExternalInput").ap()
    ssk = nc.dram_tensor("ssk", [128, T], bf16, kind="ExternalInput").ap()
    triu = nc.dram_tensor("triu", [128, 128], bf16, kind="ExternalInput").ap()
    part = nc.dram_tensor("part", [BT, COUT], bf16, kind="ExternalOutput").ap()

    with ExitStack() as ctx:
        tc = ctx.enter_context(tile.TileContext(nc))
        const_pool = ctx.enter_context(tc.tile_pool(name="const", bufs=1))
        xc_pool = ctx.enter_context(tc.tile_pool(name="xc", bufs=2))
        rot_pool = ctx.enter_context(tc.tile_pool(name="rot", bufs=4 * HPC))
        swap_pool = ctx.enter_context(tc.tile_pool(name="swap", bufs=3))
        a_pool = ctx.enter_context(tc.tile_pool(name="ropeA", bufs=3))
        v_pool = ctx.enter_context(tc.tile_pool(name="vsb", bufs=2))
        u_pool = ctx.enter_context(tc.tile_pool(name="u", bufs=6))
        uacc_pool = ctx.enter_context(tc.tile_pool(name="uacc", bufs=3))
        r_pool = ctx.enter_context(tc.tile_pool(name="recip", bufs=2))
        y_pool = ctx.enter_context(tc.tile_pool(name="y", bufs=2 * HPC))
        o_pool = ctx.enter_context(tc.tile_pool(name="osb", bufs=3))
        ps_qk = ctx.enter_context(tc.tile_pool(name="psqk", bufs=2, space="PSUM"))
        ps_v = ctx.enter_context(tc.tile_pool(name="psv", bufs=1, space="PSUM"))
        ps_S = ctx.enter_context(tc.tile_pool(name="psS", bufs=2, space="PSUM"))
        ps_y = ctx.enter_context(tc.tile_pool(name="psy", bufs=2, space="PSUM"))
        ps_d = ctx.enter_context(tc.tile_pool(name="psd", bufs=1, space="PSUM"))

        # blocked view of x: [p, b, chunk, kb, t]
        xtv = xt.rearrange("p (b c kb t) -> p b c kb t", b=B, c=NCH, kb=KB)

        # ---- persistent constants ----
        # load order matters for startup latency: q/k weights + rope tables
        # feed the first matmuls; wv is needed a bit later, wo only at the
        # first output projection.
        wq_sb = const_pool.tile([128, KB * F], bf16, tag="wq")
        nc.sync.dma_start(wq_sb[:], wq)
        xc0 = xc_pool.tile([128, KB * CH], bf16, tag="xc", name="xc0")
        nc.sync.dma_start(
            xc0[:].rearrange("p (kb t) -> p kb t", kb=KB), xtv[:, 0, 0]
        )
        wk_sb = const_pool.tile([128, KB * F], bf16, tag="wk")
        nc.sync.dma_start(wk_sb[:], wk)
        # rope tables: cos is duplicated across partition halves (2 small DMAs
        # from the same 64-row source); the 1/sqrt(D)-scaled q-tables are
        # derived on the scalar engine instead of DMA'd — cuts the startup
        # DMA wave from 2MB of tables to 0.75MB
        cck_sb = const_pool.tile([128, T], bf16, tag="cck")
        nc.sync.dma_start(cck_sb[0:64, :], cos64)
        nc.sync.dma_start(cck_sb[64:128, :], cos64)
        ssk_sb = const_pool.tile([128, T], bf16, tag="ssk")
        nc.sync.dma_start(ssk_sb[:], ssk)
        rope_scale = 1.0 / math.sqrt(D)
        ccq_sb = const_pool.tile([128, T], bf16, tag="ccq")
        nc.scalar.activation(ccq_sb[:], cck_sb[:], AF.Copy, scale=rope_scale)
        ssq_sb = const_pool.tile([128, T], bf16, tag="ssq")
        nc.scalar.activation(ssq_sb[:], ssk_sb[:], AF.Copy, scale=rope_scale)
        triu_sb = const_pool.tile([128, 128], bf16, tag="triu")
        nc.sync.dma_start(triu_sb[:], triu)
        ones_sb = const_pool.tile([128, 128], bf16, tag="ones128")
        nc.vector.memset(ones_sb[:], 1.0)
        # wv/wo are not needed until the first V-projection / out-projection:
        # gate their (large) DMAs behind the first q/k matmul groups via a
        # dummy same-slot tile, so they don't steal HBM bandwidth from the
        # critical wq/xc0/wk startup wave
        gv = const_pool.tile([128, KB * F], bf16, tag="wv", name="wv_gate")
        nc.vector.memset(gv[:, 0:16], 0)
        go = const_pool.tile([128, HPC * COUT], bf16, tag="wo", name="wo_gate")
        nc.vector.memset(go[:, 0:16], 0)
        wv_sb = wo_sb = None

        # ---- PE warm-up: keep the HAM clock gate busy while the initial
        # DMA wave streams in, so the first real matmuls run at 2.4 GHz ----
        junk = const_pool.tile([128, 128], bf16, tag="warmjunk")
        nc.vector.memset(junk[:], 0)
        psw = ps_S.tile([128, 128], f32, tag="psS", name="pswarm")
        for _ in range(100):
            nc.tensor.matmul(
                psw[:], junk[:], junk[:], start=True, stop=True,
                skip_group_check=True,
            )

        # Deferred out-projection units (one q-chunk of token tiles each).
        # Emitted with a one-chunk lag so the softmax-normalization chain of
        # chunk j completes while PE runs chunk j+1's attention matmuls —
        # emitting immediately would park that latency on PE's in-order path.
        pending = []

        def emit_outproj(final=False):
            ysb_, b_, j_ = pending.pop(0)
            for m in range(j_ * (CH // 128), (j_ + 1) * (CH // 128)):
                osb = o_pool.tile([128, COUT], bf16, tag="osb", name="osb")
                for nch in range(COUT // CH):
                    pso = ps_qk.tile([128, CH], f32, tag="psqk", name="pso")
                    for h in range(HPC):
                        nc.tensor.matmul(
                            pso[:],
                            ysb_[h][:, m * 128 : (m + 1) * 128],
                            wo_sb[
                                :,
                                h * COUT + nch * CH : h * COUT + (nch + 1) * CH,
                            ],
                            start=(h == 0),
                            stop=(h == HPC - 1),
                            skip_group_check=True,
                        )
                    if nch % 2 == 0:
                        nc.scalar.copy(osb[:, nch * CH : (nch + 1) * CH], pso[:])
                    else:
                        nc.vector.tensor_copy(
                            osb[:, nch * CH : (nch + 1) * CH], pso[:]
                        )
                    if final:
                        # tail: ship each 512-col chunk as its copy completes
                        nc.sync.dma_start(
                            part[
                                b_ * T + m * 128 : b_ * T + (m + 1) * 128,
                                nch * CH : (nch + 1) * CH,
                            ],
                            osb[:, nch * CH : (nch + 1) * CH],
                        )
                if not final:
                    nc.sync.dma_start(
                        part[b_ * T + m * 128 : b_ * T + (m + 1) * 128, :], osb[:]
                    )

        prefetched = {}
        for b in range(B):
            # ---- qkv projection + RoPE for this batch ----
            qrot = [rot_pool.tile([128, T], bf16, tag="rot", name=f"qrot{h}") for h in range(HPC)]
            krot = [rot_pool.tile([128, T], bf16, tag="rot", name=f"krot{h}") for h in range(HPC)]
            vsb = v_pool.tile([128, NTT * F], bf16, tag="v")
            for c in range(NCH):
                if b == 0 and c == 0:
                    xc = xc0
                elif (b, c) in prefetched:
                    xc = prefetched.pop((b, c))
                else:
                    xc = xc_pool.tile([128, KB * CH], bf16, tag="xc")
                    nc.sync.dma_start(
                        xc[:].rearrange("p (kb t) -> p kb t", kb=KB), xtv[:, b, c]
                    )
                for h in range(HPC):
                    for wsb, ccs, sss, dst in (
                        (wq_sb, ccq_sb, ssq_sb, qrot[h]),
                        (wk_sb, cck_sb, ssk_sb, krot[h]),
                    ):
                        ps = ps_qk.tile([128, CH], f32, tag="psqk")
                        for kb in range(KB):
                            nc.tensor.matmul(
                                ps[:],
                                wsb[:, kb * F + h * D : kb * F + (h + 1) * D],
                                xc[:, kb * CH : (kb + 1) * CH],
                                start=(kb == 0),
                                stop=(kb == KB - 1),
                            )
                        # RoPE: rot = ps * cc + halfswap(ps) * ss
                        sw = swap_pool.tile([128, CH], bf16, tag="swap")
                        nc.vector.tensor_copy(sw[0:64, :], ps[64:128, :])
                        nc.vector.tensor_copy(sw[64:128, :], ps[0:64, :])
                        A = a_pool.tile([128, CH], f32, tag="ropeA")
                        nc.vector.tensor_mul(
                            A[:], ps[:], ccs[:, c * CH : (c + 1) * CH]
                        )
                        Bt = a_pool.tile([128, CH], bf16, tag="ropeB")
                        nc.vector.tensor_mul(
                            Bt[:], sw[:], sss[:, c * CH : (c + 1) * CH]
                        )
                        nc.vector.tensor_add(
                            dst[:, c * CH : (c + 1) * CH], A[:], Bt[:]
                        )
                    if b == 0 and c == 0 and h == 0:
                        # release the wv/wo DMA gates now that the critical
                        # startup DMAs have been consumed
                        for g in (gv, go):
                            psg = ps_S.tile([16, 16], f32, tag="psS", name="psg")
                            nc.tensor.matmul(
                                psg[:], g[:, 0:16], g[:, 0:16],
                                start=True, stop=True, skip_group_check=True,
                            )
                        wv_sb = const_pool.tile([128, KB * F], bf16, tag="wv")
                        nc.sync.dma_start(wv_sb[:], wv)
                        wo_sb = const_pool.tile([128, HPC * COUT], bf16, tag="wo")
                        nc.sync.dma_start(wo_sb[:], wo)
                # V in [token part, feature free] layout: x-blocks stationary
                for tm in range(CH // 128):
                    psv = ps_v.tile([128, F], f32, tag="psv")
                    for kb in range(KB):
                        nc.tensor.matmul(
                            psv[:],
                            xc[:, kb * CH + tm * 128 : kb * CH + tm * 128 + 128],
                            wv_sb[:, kb * F : (kb + 1) * F],
                            start=(kb == 0),
                            stop=(kb == KB - 1),
                        )
                    tt = c * (CH // 128) + tm
                    nc.scalar.copy(vsb[:, tt * F : (tt + 1) * F], psv[:])

            # prefetch the next batch's first two x-chunks during attention,
            # ahead of this batch's output-DMA wave (the xc buffers are free
            # once this batch's projection has consumed them)
            if b + 1 < B:
                for c in (0, 1):
                    xcp = xc_pool.tile([128, KB * CH], bf16, tag="xc")
                    nc.sync.dma_start(
                        xcp[:].rearrange("p (kb t) -> p kb t", kb=KB),
                        xtv[:, b + 1, c],
                    )
                    prefetched[(b + 1, c)] = xcp

            # ---- attention per head, out-projection interleaved per q-chunk ----
            ysb = [y_pool.tile([128, T], bf16, tag="y", name=f"ysb{h}") for h in range(HPC)]
            for j in range(NCH):
                per_head = []
                # PV matmuls run one k-tile behind the scores matmuls, so exp
                # always has at least one full matmul of slack before its
                # consumer issues on the in-order PE queue
                pvq = []

                def flush_pv():
                    while pvq:
                        nc.tensor.matmul(**pvq.pop(0))

                for h in range(HPC):
                    psy = ps_y.tile([128, CH], f32, tag="psy")
                    uacc = uacc_pool.tile([128, CH], bf16, tag="uacc")
                    nkb = (CH // 128) * (j + 1)  # causal: k-tiles <= q-chunk end
                    for kb in range(nkb):
                        c0 = max(0, kb * 128 - j * CH)
                        psS = ps_S.tile([128, CH], f32, tag="psS")
                        nc.tensor.matmul(
                            psS[:, c0:CH],
                            krot[h][:, kb * 128 : (kb + 1) * 128],
                            qrot[h][:, j * CH + c0 : (j + 1) * CH],
                            start=True,
                            stop=True,
                        )
                        flush_pv()
                        if kb == 0:
                            # exp lands straight in the accumulator; it doubles
                            # as the PV rhs for this k-tile (no copy needed)
                            U = uacc
                        else:
                            U = u_pool.tile([128, CH], bf16, tag="u")
                        nc.scalar.activation(U[:, c0:CH], psS[:, c0:CH], AF.Exp)
                        if kb * 128 >= j * CH:
                            # diagonal 128x128 block: zero out k > q after exp
                            nc.vector.tensor_mul(
                                U[:, c0 : c0 + 128],
                                U[:, c0 : c0 + 128],
                                triu_sb[:],
                            )
                        if kb > 0:
                            nc.vector.tensor_add(
                                uacc[:, c0:CH], uacc[:, c0:CH], U[:, c0:CH]
                            )
                        pvq.append(
                            dict(
                                out=psy[:, c0:CH],
                                lhsT=vsb[:, kb * F + h * D : kb * F + (h + 1) * D],
                                rhs=U[:, c0:CH],
                                start=(kb == 0),
                                stop=(kb == nkb - 1),
                                skip_group_check=True,
                            )
                        )
                    per_head.append((psy, uacc))

                def emit_norm(h):
                    # all-ones lhsT sums U_acc over partitions and replicates
                    # the denominator to every output partition in one matmul
                    psy, uacc = per_head[h]
                    psR = ps_d.tile([128, CH], f32, tag="psd")
                    nc.tensor.matmul(
                        psR[:], ones_sb[:], uacc[:], start=True, stop=True,
                        skip_group_check=True,
                    )
                    flush_pv()
                    rr = r_pool.tile([128, CH], f32, tag="recip")
                    nc.vector.reciprocal_approx_fast(rr[:], psR[:])
                    nc.vector.tensor_mul(
                        ysb[h][:, j * CH : (j + 1) * CH], psy[:], rr[:]
                    )

                # h=0's U_acc finished while PE ran h=1's attention matmuls;
                # the last head's norm is emitted after the deferred
                # out-projection so PE never waits on the DVE add chain.
                emit_norm(0)
                pending.append((ysb, b, j))
                if len(pending) > 1:
                    emit_outproj()
                for h in range(1, HPC):
                    emit_norm(h)

        while pending:
            emit_outproj(final=(len(pending) == 1))

    nc.compile()
    return nc


def make_host_inputs(x, cos, sin, Wqkv, Wout, H, n_cores):
    """Shard + precompute the per-core device input maps (numpy, host side)."""
    bf16 = ml_dtypes.bfloat16
    B, T, C = x.shape
    D = C // H
    HPC = H // n_cores
    COUT = Wout.shape[1]
    KB = C // 128
    NCH = T // _CH

    # block x into [p, b, chunk, kb, t] so each chunk DMA is 128 contiguous
    # 16KB runs (one per partition)
    x2 = np.asarray(x).reshape(B * T, C).T.astype(bf16)  # [C, B*T]
    xt = np.ascontiguousarray(
        x2.reshape(KB, 128, B, NCH, _CH).transpose(1, 2, 3, 0, 4)
    ).reshape(128, B * NCH * KB * _CH)

    def blk(w):
        # [C, F_] -> [128, KB*F_] partition-major weight blocks
        F_ = w.shape[1]
        return np.ascontiguousarray(
            w.reshape(KB, 128, F_).transpose(1, 0, 2).reshape(128, KB * F_)
        ).astype(bf16)

    # deinterleave permutation within each head: [0,2,4,...,1,3,5,...]
    perm = np.concatenate([np.arange(0, D, 2), np.arange(1, D, 2)])
    Wq = Wqkv[:, 0:C].reshape(C, H, D)[:, :, perm]
    Wk = Wqkv[:, C : 2 * C].reshape(C, H, D)[:, :, perm]
    Wv = Wqkv[:, 2 * C : 3 * C].reshape(C, H, D)

    cos64 = np.ascontiguousarray(cos.T).astype(bf16)  # [D/2, T]
    ssk = np.concatenate([-sin.T, sin.T], axis=0).astype(bf16)  # [D, T]

    tri = np.triu(np.ones((128, 128), dtype=np.float32)).astype(bf16)

    in_maps = []
    for core in range(n_cores):
        hs = slice(core * HPC, (core + 1) * HPC)
        in_maps.append(
            {
                "xt": xt,
                "wq": blk(Wq[:, hs, :].reshape(C, HPC * D)),
                "wk": blk(Wk[:, hs, :].reshape(C, HPC * D)),
                "wv": blk(Wv[:, hs, :].reshape(C, HPC * D)),
                "wo": np.ascontiguousarray(
                    Wout[core * HPC * D : (core + 1) * HPC * D, :]
                    .reshape(HPC, 128, COUT)
                    .transpose(1, 0, 2)
                    .reshape(128, HPC * COUT)
                ).astype(bf16),
                "cos64": cos64,
                "ssk": ssk,
                "triu": tri,
            }
        )
    return in_maps


_PROGRAM_CACHE = {}


def kernel(x, cos, sin, Wqkv, Wout):
    global LAST_RESULT
    from concourse.bass_utils import run_bass_kernel_spmd

    x = np.asarray(x, dtype=np.float32)
    cos = np.asarray(cos, dtype=np.float32)
    sin = np.asarray(sin, dtype=np.float32)
    Wqkv = np.asarray(Wqkv, dtype=np.float32)
    Wout = np.asarray(Wout, dtype=np.float32)

    B, T, C = x.shape
    H = _H
    COUT = Wout.shape[1]
    n_cores = 8
    HPC = H // n_cores

    key = (B, T, C, COUT, HPC, n_cores)
    if key not in _PROGRAM_CACHE:
        _PROGRAM_CACHE[key] = build_program(B, T, C, COUT, HPC, n_cores)
    nc = _PROGRAM_CACHE[key]

    in_maps = make_host_inputs(x, cos, sin, Wqkv, Wout, H, n_cores)
    res = run_bass_kernel_spmd(
        nc, in_maps, core_ids=list(range(n_cores)), trace=TRACE
    )
    LAST_RESULT = res

    out = np.zeros((B * T, COUT), dtype=np.float32)
    for r in res.results:
        out += np.asarray(r["part"], dtype=np.float32)
    return out.reshape(B, T, COUT)



# revision 33
# speedup vs baseline: 1.0047x; 1.0047x over previous
"""Trainium2 Bass kernel for causal self-attention (B=4, T=2048, C=2048, H=16).

Sharding: 16 heads across 8 cores (2 heads/core), tensor-parallel column split
of Wqkv and row split of Wout; host sums the 8 row-parallel partial outputs.

Device-side layout strategy (everything "transposed", token index in the free
dimension), which makes every matmul land in its natural layout with zero
on-chip transposes:
  qT/kT   [D=128 part, T free]   = Wq_h^T @ x  (m-tile of the qkv projection)
  V       [T part, D free]       computed with x-blocks as the stationary operand
  S^T     [k part, q free]       = kT-block^T-contraction (lhsT=kT blk, rhs=qT)
  U = exp(S^T), causal blocks skipped entirely, diagonal blocks masked via a
      triangular 0/1 multiply after exp
  y^T     [D part, q free]       = sum_kb V_blk.T @ U_blk  (PSUM accumulate)
  U_acc   [128, q]               = sum_kb U_blk            (DVE adds, bf16)
  denomR  [128 part, q free]     = ones128.T @ U_acc  (one matmul per (j,h);
            the all-ones lhsT replicates the denominator to every partition,
            so no gpsimd broadcast is needed)
  y_norm  = y^T * reciprocal_approx_fast(denomR)
  partial [tokens part, Cout]    lhsT = y^T block, rhs = Wout rows, emitted
            with a one-q-chunk lag so PE never waits on the softmax
            normalization chain; junk warm-up matmuls cover the initial DMA
            wave so HAM runs the PE at 2.4 GHz from the first real matmul

RoPE is applied in [D, T] layout: the q/k columns of Wqkv are permuted on the
host so rotation pairs land at partitions (i, i+64) ("deinterleaved"); the
half-swap is two 64-partition DVE copies, then 3 elementwise ops against
host-precomputed [128, T] cos/sin tables (the 1/sqrt(D) scale is folded into
the q tables). Scores are invariant to any fixed permutation of head dims
applied to both q and k, so the reference is reproduced exactly.
"""

import math

import numpy as np
import ml_dtypes

# Module-level knobs (test harness may set TRACE=True to capture a profile).
TRACE = False
LAST_RESULT = None  # BassKernelResults of the last run (for profiling)

_B, _T, _C, _H = 4, 2048, 2048, 16
_D = 128
_CH = 512  # free-dim chunk (one PSUM bank of fp32)


def build_program(B, T, C, COUT, HPC, n_cores=8):
    """Build the per-core Bass/Tile program (identical on all cores)."""
    import concourse.bass as bass  # noqa: F401
    import concourse.mybir as mybir
    import concourse.tile as tile
    from concourse import bacc
    from contextlib import ExitStack

    dt = mybir.dt
    f32, bf16 = dt.float32, dt.bfloat16
    D, CH = _D, _CH
    KB = C // 128          # contraction blocks for the projections
    NCH = T // CH          # 512-token chunks per batch (also q-chunks)
    NTT = T // 128         # 128-token tiles per batch (also k-tiles)
    BT = B * T
    F = HPC * D            # per-core head feature width
    AF = mybir.ActivationFunctionType

    nc = bacc.Bacc(
        "TRN2", target_bir_lowering=False, debug=False, num_devices=n_cores
    )

    # all bulk inputs are pre-blocked on the host into partition-major layout
    # so every DMA moves long contiguous runs (8-16KB descriptors instead of
    # 0.5-1KB ones; the DMA engines are descriptor-rate-limited)
    xt = nc.dram_tensor("xt", [128, B * NCH * KB * CH], bf16, kind="ExternalInput").ap()
    wq = nc.dram_tensor("wq", [128, KB * F], bf16, kind="ExternalInput").ap()
    wk = nc.dram_tensor("wk", [128, KB * F], bf16, kind="ExternalInput").ap()
    wv = nc.dram_tensor("wv", [128, KB * F], bf16, kind="ExternalInput").ap()
    wo = nc.dram_tensor("wo", [128, HPC * COUT], bf16, kind="ExternalInput").ap()
    cos64 = nc.dram_tensor("cos64", [64, T], bf16, kind="ExternalInput").ap()
    ssk = nc.dram_tensor("ssk", [128, T], bf16, kind="ExternalInput").ap()
    triu = nc.dram_tensor("triu", [128, 128], bf16, kind="ExternalInput").ap()
    part = nc.dram_tensor("part", [BT, COUT], bf16, kind="ExternalOutput").ap()

    with ExitStack() as ctx:
        tc = ctx.enter_context(tile.TileContext(nc))
        const_pool = ctx.enter_context(tc.tile_pool(name="const", bufs=1))
        xc_pool = ctx.enter_context(tc.tile_pool(name="xc", bufs=2))
        rot_pool = ctx.enter_context(tc.tile_pool(name="rot", bufs=4 * HPC))
        swap_pool = ctx.enter_context(tc.tile_pool(name="swap", bufs=3))
        a_pool = ctx.enter_context(tc.tile_pool(name="ropeA", bufs=3))
        v_pool = ctx.enter_context(tc.tile_pool(name="vsb", bufs=2))
        u_pool = ctx.enter_context(tc.tile_pool(name="u", bufs=6))
        uacc_pool = ctx.enter_context(tc.tile_pool(name="uacc", bufs=3))
        r_pool = ctx.enter_context(tc.tile_pool(name="recip", bufs=2))
        y_pool = ctx.enter_context(tc.tile_pool(name="y", bufs=2 * HPC))
        o_pool = ctx.enter_context(tc.tile_pool(name="osb", bufs=3))
        ps_qk = ctx.enter_context(tc.tile_pool(name="psqk", bufs=2, space="PSUM"))
        ps_v = ctx.enter_context(tc.tile_pool(name="psv", bufs=1, space="PSUM"))
        ps_S = ctx.enter_context(tc.tile_pool(name="psS", bufs=2, space="PSUM"))
        ps_y = ctx.enter_context(tc.tile_pool(name="psy", bufs=2, space="PSUM"))
        ps_d = ctx.enter_context(tc.tile_pool(name="psd", bufs=1, space="PSUM"))

        # blocked view of x: [p, b, chunk, kb, t]
        xtv = xt.rearrange("p (b c kb t) -> p b c kb t", b=B, c=NCH, kb=KB)

        # ---- persistent constants ----
        # load order matters for startup latency: q/k weights + rope tables
        # feed the first matmuls; wv is needed a bit later, wo only at the
        # first output projection.
        wq_sb = const_pool.tile([128, KB * F], bf16, tag="wq")
        nc.sync.dma_start(wq_sb[:], wq)
        xc0 = xc_pool.tile([128, KB * CH], bf16, tag="xc", name="xc0")
        nc.sync.dma_start(
            xc0[:].rearrange("p (kb t) -> p kb t", kb=KB), xtv[:, 0, 0]
        )
        wk_sb = const_pool.tile([128, KB * F], bf16, tag="wk")
        nc.sync.dma_start(wk_sb[:], wk)
        # rope tables: cos is duplicated across partition halves (2 small DMAs
        # from the same 64-row source); the 1/sqrt(D)-scaled q-tables are
        # derived on the scalar engine instead of DMA'd — cuts the startup
        # DMA wave from 2MB of tables to 0.75MB
        cck_sb = const_pool.tile([128, T], bf16, tag="cck")
        nc.sync.dma_start(cck_sb[0:64, :], cos64)
        nc.sync.dma_start(cck_sb[64:128, :], cos64)
        ssk_sb = const_pool.tile([128, T], bf16, tag="ssk")
        nc.sync.dma_start(ssk_sb[:], ssk)
        rope_scale = 1.0 / math.sqrt(D)
        ccq_sb = const_pool.tile([128, T], bf16, tag="ccq")
        nc.scalar.activation(ccq_sb[:], cck_sb[:], AF.Copy, scale=rope_scale)
        ssq_sb = const_pool.tile([128, T], bf16, tag="ssq")
        nc.scalar.activation(ssq_sb[:], ssk_sb[:], AF.Copy, scale=rope_scale)
        triu_sb = const_pool.tile([128, 128], bf16, tag="triu")
        nc.sync.dma_start(triu_sb[:], triu)
        ones_sb = const_pool.tile([128, 128], bf16, tag="ones128")
        nc.vector.memset(ones_sb[:], 1.0)
        # wv/wo are not needed until the first V-projection / out-projection:
        # gate their (large) DMAs behind the first q/k matmul groups via a
        # dummy same-slot tile, so they don't steal HBM bandwidth from the
        # critical wq/xc0/wk startup wave
        gv = const_pool.tile([128, KB * F], bf16, tag="wv", name="wv_gate")
        nc.vector.memset(gv[:, 0:16], 0)
        go = const_pool.tile([128, HPC * COUT], bf16, tag="wo", name="wo_gate")
        nc.vector.memset(go[:, 0:16], 0)
        wv_sb = wo_sb = None

        # ---- PE warm-up: keep the HAM clock gate busy while the initial
        # DMA wave streams in, so the first real matmuls run at 2.4 GHz ----
        junk = const_pool.tile([128, 128], bf16, tag="warmjunk")
        nc.vector.memset(junk[:], 0)
        psw = ps_S.tile([128, 128], f32, tag="psS", name="pswarm")
        for _ in range(100):
            nc.tensor.matmul(
                psw[:], junk[:], junk[:], start=True, stop=True,
                skip_group_check=True,
            )

        # Deferred out-projection units (one q-chunk of token tiles each).
        # Emitted with a one-chunk lag so the softmax-normalization chain of
        # chunk j completes while PE runs chunk j+1's attention matmuls —
        # emitting immediately would park that latency on PE's in-order path.
        pending = []

        def emit_outproj(final=False):
            ysb_, b_, j_ = pending.pop(0)
            for m in range(j_ * (CH // 128), (j_ + 1) * (CH // 128)):
                osb = o_pool.tile([128, COUT], bf16, tag="osb", name="osb")
                for nch in range(COUT // CH):
                    pso = ps_qk.tile([128, CH], f32, tag="psqk", name="pso")
                    for h in range(HPC):
                        nc.tensor.matmul(
                            pso[:],
                            ysb_[h][:, m * 128 : (m + 1) * 128],
                            wo_sb[
                                :,
                                h * COUT + nch * CH : h * COUT + (nch + 1) * CH,
                            ],
                            start=(h == 0),
                            stop=(h == HPC - 1),
                            skip_group_check=True,
                        )
                    if nch % 2 == 0:
                        nc.scalar.copy(osb[:, nch * CH : (nch + 1) * CH], pso[:])
                    else:
                        nc.vector.tensor_copy(
                            osb[:, nch * CH : (nch + 1) * CH], pso[:]
                        )
                    if final:
                        # tail: ship each 512-col chunk as its copy completes
                        nc.sync.dma_start(
                            part[
                                b_ * T + m * 128 : b_ * T + (m + 1) * 128,
                                nch * CH : (nch + 1) * CH,
                            ],
                            osb[:, nch * CH : (nch + 1) * CH],
                        )
                if not final:
                    nc.sync.dma_start(
                        part[b_ * T + m * 128 : b_ * T + (m + 1) * 128, :], osb[:]
                    )

        prefetched = {}
        for b in range(B):
            # ---- qkv projection + RoPE for this batch ----
            qrot = [rot_pool.tile([128, T], bf16, tag="rot", name=f"qrot{h}") for h in range(HPC)]
            krot = [rot_pool.tile([128, T], bf16, tag="rot", name=f"krot{h}") for h in range(HPC)]
            vsb = v_pool.tile([128, NTT * F], bf16, tag="v")
            for c in range(NCH):
                if b == 0 and c == 0:
                    xc = xc0
                elif (b, c) in prefetched:
                    xc = prefetched.pop((b, c))
                else:
                    xc = xc_pool.tile([128, KB * CH], bf16, tag="xc")
                    nc.sync.dma_start(
                        xc[:].rearrange("p (kb t) -> p kb t", kb=KB), xtv[:, b, c]
                    )
                for h in range(HPC):
                    for wsb, ccs, sss, dst in (
                        (wq_sb, ccq_sb, ssq_sb, qrot[h]),
                        (wk_sb, cck_sb, ssk_sb, krot[h]),
                    ):
                        ps = ps_qk.tile([128, CH], f32, tag="psqk")
                        for kb in range(KB):
                            nc.tensor.matmul(
                                ps[:],
                                wsb[:, kb * F + h * D : kb * F + (h + 1) * D],
                                xc[:, kb * CH : (kb + 1) * CH],
                                start=(kb == 0),
                                stop=(kb == KB - 1),
                            )
                        # RoPE: rot = ps * cc + halfswap(ps) * ss
                        sw = swap_pool.tile([128, CH], bf16, tag="swap")
                        nc.vector.tensor_copy(sw[0:64, :], ps[64:128, :])
                        nc.vector.tensor_copy(sw[64:128, :], ps[0:64, :])
                        A = a_pool.tile([128, CH], f32, tag="ropeA")
                        nc.vector.tensor_mul(
                            A[:], ps[:], ccs[:, c * CH : (c + 1) * CH]
                        )
                        Bt = a_pool.tile([128, CH], bf16, tag="ropeB")
                        nc.vector.tensor_mul(
                            Bt[:], sw[:], sss[:, c * CH : (c + 1) * CH]
                        )
                        nc.vector.tensor_add(
                            dst[:, c * CH : (c + 1) * CH], A[:], Bt[:]
                        )
                    if b == 0 and c == 0 and h == 0:
                        # release the wv/wo DMA gates now that the critical
                        # startup DMAs have been consumed
                        for g in (gv, go):
                            psg = ps_S.tile([16, 16], f32, tag="psS", name="psg")
                            nc.tensor.matmul(
                                psg[:], g[:, 0:16], g[:, 0:16],
                                start=True, stop=True, skip_group_check=True,
                            )
                        wv_sb = const_pool.tile([128, KB * F], bf16, tag="wv")
                        nc.sync.dma_start(wv_sb[:], wv)
                        wo_sb = const_pool.tile([128, HPC * COUT], bf16, tag="wo")
                        nc.sync.dma_start(wo_sb[:], wo)
                # V in [token part, feature free] layout: x-blocks stationary
                for tm in range(CH // 128):
                    psv = ps_v.tile([128, F], f32, tag="psv")
                    for kb in range(KB):
                        nc.tensor.matmul(
                            psv[:],
                            xc[:, kb * CH + tm * 128 : kb * CH + tm * 128 + 128],
                            wv_sb[:, kb * F : (kb + 1) * F],
                            start=(kb == 0),
                            stop=(kb == KB - 1),
                        )
                    tt = c * (CH // 128) + tm
                    nc.scalar.copy(vsb[:, tt * F : (tt + 1) * F], psv[:])

            # prefetch the next batch's first two x-chunks during attention,
            # ahead of this batch's output-DMA wave (the xc buffers are free
            # once this batch's projection has consumed them)
            if b + 1 < B:
                for c in (0, 1):
                    xcp = xc_pool.tile([128, KB * CH], bf16, tag="xc")
                    nc.sync.dma_start(
                        xcp[:].rearrange("p (kb t) -> p kb t", kb=KB),
                        xtv[:, b + 1, c],
                    )
                    prefetched[(b + 1, c)] = xcp

            # ---- attention per head, out-projection interleaved per q-chunk ----
            ysb = [y_pool.tile([128, T], bf16, tag="y", name=f"ysb{h}") for h in range(HPC)]
            for j in range(NCH):
                per_head = []
                for h in range(HPC):
                    psy = ps_y.tile([128, CH], f32, tag="psy")
                    uacc = uacc_pool.tile([128, CH], bf16, tag="uacc")
                    nkb = (CH // 128) * (j + 1)  # causal: k-tiles <= q-chunk end
                    for kb in range(nkb):
                        c0 = max(0, kb * 128 - j * CH)
                        psS = ps_S.tile([128, CH], f32, tag="psS")
                        nc.tensor.matmul(
                            psS[:, c0:CH],
                            krot[h][:, kb * 128 : (kb + 1) * 128],
                            qrot[h][:, j * CH + c0 : (j + 1) * CH],
                            start=True,
                            stop=True,
                        )
                        if kb == 0:
                            # exp lands straight in the accumulator; it doubles
                            # as the PV rhs for this k-tile (no copy needed)
                            U = uacc
                        else:
                            U = u_pool.tile([128, CH], bf16, tag="u")
                        nc.scalar.activation(U[:, c0:CH], psS[:, c0:CH], AF.Exp)
                        if kb * 128 >= j * CH:
                            # diagonal 128x128 block: zero out k > q after exp
                            nc.vector.tensor_mul(
                                U[:, c0 : c0 + 128],
                                U[:, c0 : c0 + 128],
                                triu_sb[:],
                            )
                        if kb > 0:
                            nc.vector.tensor_add(
                                uacc[:, c0:CH], uacc[:, c0:CH], U[:, c0:CH]
                            )
                        nc.tensor.matmul(
                            psy[:, c0:CH],
                            vsb[:, kb * F + h * D : kb * F + (h + 1) * D],
                            U[:, c0:CH],
                            start=(kb == 0),
                            stop=(kb == nkb - 1),
                            skip_group_check=True,
                        )
                    per_head.append((psy, uacc))

                def emit_norm(h):
                    # all-ones lhsT sums U_acc over partitions and replicates
                    # the denominator to every output partition in one matmul
                    psy, uacc = per_head[h]
                    psR = ps_d.tile([128, CH], f32, tag="psd")
                    nc.tensor.matmul(
                        psR[:], ones_sb[:], uacc[:], start=True, stop=True,
                        skip_group_check=True,
                    )
                    rr = r_pool.tile([128, CH], f32, tag="recip")
                    nc.vector.reciprocal_approx_fast(rr[:], psR[:])
                    nc.vector.tensor_mul(
                        ysb[h][:, j * CH : (j + 1) * CH], psy[:], rr[:]
                    )

                # h=0's U_acc finished while PE ran h=1's attention matmuls;
                # the last head's norm is emitted after the deferred
                # out-projection so PE never waits on the DVE add chain.
                emit_norm(0)
                pending.append((ysb, b, j))
                if len(pending) > 1:
                    emit_outproj()
                for h in range(1, HPC):
                    emit_norm(h)

        while pending:
            emit_outproj(final=(len(pending) == 1))

    nc.compile()
    return nc


def make_host_inputs(x, cos, sin, Wqkv, Wout, H, n_cores):
    """Shard + precompute the per-core device input maps (numpy, host side)."""
    bf16 = ml_dtypes.bfloat16
    B, T, C = x.shape
    D = C // H
    HPC = H // n_cores
    COUT = Wout.shape[1]
    KB = C // 128
    NCH = T // _CH

    # block x into [p, b, chunk, kb, t] so each chunk DMA is 128 contiguous
    # 16KB runs (one per partition)
    x2 = np.asarray(x).reshape(B * T, C).T.astype(bf16)  # [C, B*T]
    xt = np.ascontiguousarray(
        x2.reshape(KB, 128, B, NCH, _CH).transpose(1, 2, 3, 0, 4)
    ).reshape(128, B * NCH * KB * _CH)

    def blk(w):
        # [C, F_] -> [128, KB*F_] partition-major weight blocks
        F_ = w.shape[1]
        return np.ascontiguousarray(
            w.reshape(KB, 128, F_).transpose(1, 0, 2).reshape(128, KB * F_)
        ).astype(bf16)

    # deinterleave permutation within each head: [0,2,4,...,1,3,5,...]
    perm = np.concatenate([np.arange(0, D, 2), np.arange(1, D, 2)])
    Wq = Wqkv[:, 0:C].reshape(C, H, D)[:, :, perm]
    Wk = Wqkv[:, C : 2 * C].reshape(C, H, D)[:, :, perm]
    Wv = Wqkv[:, 2 * C : 3 * C].reshape(C, H, D)

    cos64 = np.ascontiguousarray(cos.T).astype(bf16)  # [D/2, T]
    ssk = np.concatenate([-sin.T, sin.T], axis=0).astype(bf16)  # [D, T]

    tri = np.triu(np.ones((128, 128), dtype=np.float32)).astype(bf16)

    in_maps = []
    for core in range(n_cores):
        hs = slice(core * HPC, (core + 1) * HPC)
        in_maps.append(
            {
                "xt": xt,
                "wq": blk(Wq[:, hs, :].reshape(C, HPC * D)),
                "wk": blk(Wk[:, hs, :].reshape(C, HPC * D)),
                "wv": blk(Wv[:, hs, :].reshape(C, HPC * D)),
                "wo": np.ascontiguousarray(
                    Wout[core * HPC * D : (core + 1) * HPC * D, :]
                    .reshape(HPC, 128, COUT)
                    .transpose(1, 0, 2)
                    .reshape(128, HPC * COUT)
                ).astype(bf16),
                "cos64": cos64,
                "ssk": ssk,
                "triu": tri,
            }
        )
    return in_maps


_PROGRAM_CACHE = {}


def kernel(x, cos, sin, Wqkv, Wout):
    global LAST_RESULT
    from concourse.bass_utils import run_bass_kernel_spmd

    x = np.asarray(x, dtype=np.float32)
    cos = np.asarray(cos, dtype=np.float32)
    sin = np.asarray(sin, dtype=np.float32)
    Wqkv = np.asarray(Wqkv, dtype=np.float32)
    Wout = np.asarray(Wout, dtype=np.float32)

    B, T, C = x.shape
    H = _H
    COUT = Wout.shape[1]
    n_cores = 8
    HPC = H // n_cores

    key = (B, T, C, COUT, HPC, n_cores)
    if key not in _PROGRAM_CACHE:
        _PROGRAM_CACHE[key] = build_program(B, T, C, COUT, HPC, n_cores)
    nc = _PROGRAM_CACHE[key]

    in_maps = make_host_inputs(x, cos, sin, Wqkv, Wout, H, n_cores)
    res = run_bass_kernel_spmd(
        nc, in_maps, core_ids=list(range(n_cores)), trace=TRACE
    )
    LAST_RESULT = res

    out = np.zeros((B * T, COUT), dtype=np.float32)
    for r in res.results:
        out += np.asarray(r["part"], dtype=np.float32)
    return out.reshape(B, T, COUT)

